# revision 53
# baseline (speedup 1.0000x reference)
"""Trainium2 Bass kernel for MultiHeadCrossAttention (B=8,N=8,Q=128,K=1024,D=512,H=8).

Sharding: data-parallel over batch B — core i handles batch i.

Key optimization: key_mask is known on the host, so invalid keys are compacted
away before the kernel runs — each batch keeps only its valid keys (~520 of
1024), zero-padded to KP (multiple of 128). All K-dependent work (K/V
projections, QK logits, softmax, AV) shrinks by ~KP/K. Padding keys carry
kv=0 and expb=0, so they contribute exactly nothing.

Per-core dataflow (all matmuls bf16 on TensorE, f32 PSUM accumulate):
  - host stages transposed bf16 activations (kvT [D, N*KP], qT [D, N*Q]) and
    transposed bf16 weights; SCALE folded into Wq/bq; Gaussian distance bias +
    key mask folded into a precomputed multiplicative exp-bias table [Q, KP];
    V-bias folded into the output bias (softmax weights sum to 1, so bv adds
    the constant bv@Wo.T to the output).
  - Q-proj once up front -> qTp [j, m] (transposed layout, heads on partitions)
  - per step n: K-proj -> kT [j, k] (transposed), V-proj -> v [k, j] (natural)
  - per head pair: logits = qT'^T @ kT (PSUM, natural [q, k]); exp on ScalarE;
    multiply by exp-bias with fused row-sum accumulation on VectorE; rowsum
    reciprocal via ScalarE LUT; per-head normalize on VectorE; DMA-xbar
    transpose attn -> [k, q]; AV accumulated over k-chunks (PSUM evacuated by
    ScalarE).
  - out-proj: 4 accumulating matmuls, bias added during final evacuation.
  - latency-critical DMAs (weights, q, first kv, output) ride SP's HWDGE
    queues; steady-state kv loads use Pool's SWDGE.
"""

import numpy as np
import ml_dtypes

B, N, Q, K, D, H = 8, 8, 128, 1024, 512, 8
HD = D // H
SCALE = HD ** -0.5
SIGMA2 = max(0.35 * 0.35, 1e-6)
NCORES = 8

_BF16 = ml_dtypes.bfloat16

_CACHE = {}


def _kp_from_mask(key_mask):
    """(KP, KW): KP = padded tile width (mult of 128, for vt/transpose
    chunking); KW = compute width (mult of 64) actually covered by
    K-proj/QK/exp. Padding keys have kv=0 and expb=0."""
    cnt = int(np.asarray(key_mask).sum(axis=1).max())
    kw = max(128, min(K, -(-cnt // 64) * 64))
    kp = -(-kw // 128) * 128
    return kp, kw


def _kblocks(KP):
    """Split [0, KP) into PSUM-bank-sized column blocks (<=512)."""
    out, st = [], 0
    while st < KP:
        sz = min(512, KP - st)
        out.append((st, sz))
        st += sz
    return out


def _build_program(repeat=1, KP=640, KW=576):
    import concourse.bass as bass
    import concourse.mybir as mybir
    import concourse.tile as tile
    from concourse import bacc

    f32 = mybir.dt.float32
    bf16 = mybir.dt.bfloat16
    AF = mybir.ActivationFunctionType
    ALU = mybir.AluOpType

    NCH = KP // 128          # 128-key chunks (vt / transposed-attn chunking)
    KB = _kblocks(KW)        # QK blocks (PSUM-bank aligned: <=512 each)
    # K-proj blocks: balanced so no block is so narrow that its LDWEIGHTS
    # (~107ns for a 128-col stationary) exceeds the stream time on HW.
    # kt is SBUF (no bank constraint), so equal 64-multiples work.
    nbk = -(-KW // 512)
    bsz = (KW // nbk + 63) // 64 * 64
    KBP, st = [], 0
    while st < KW:
        sz = min(bsz, KW - st)
        KBP.append((st, sz))
        st += sz

    nc = bacc.Bacc("TRN2", target_bir_lowering=False, debug=False,
                   num_devices=NCORES)

    kvT_h = nc.declare_dram_parameter("kvT", [D, N * KP], bf16, isOutput=False)
    qT_h = nc.declare_dram_parameter("qT", [D, N * Q], bf16, isOutput=False)
    wq_h = nc.declare_dram_parameter("wqT", [D, D], bf16, isOutput=False)
    wk_h = nc.declare_dram_parameter("wkT", [D, D], bf16, isOutput=False)
    wv_h = nc.declare_dram_parameter("wvT", [D, D], bf16, isOutput=False)
    wo_h = nc.declare_dram_parameter("woT", [D, D], bf16, isOutput=False)
    bq_h = nc.declare_dram_parameter("bq2", [128, 4], f32, isOutput=False)
    bk_h = nc.declare_dram_parameter("bk2", [128, 4], f32, isOutput=False)
    bo_h = nc.declare_dram_parameter("bob", [128, D], f32, isOutput=False)
    eb_h = nc.declare_dram_parameter("expb", [Q, KP], bf16, isOutput=False)
    out_h = nc.declare_dram_parameter("out", [N, Q, D], f32, isOutput=True)

    kvT = kvT_h.ap().rearrange("(c p) m -> p c m", p=128)   # [128, 4, N*KP]
    qT = qT_h.ap().rearrange("(c p) m -> p c m", p=128)     # [128, 4, N*Q]
    w_aps = {k: h.ap().rearrange("(c p) j -> p c j", p=128)
             for k, h in (("wq", wq_h), ("wk", wk_h), ("wv", wv_h), ("wo", wo_h))}
    out_ap = out_h.ap()

    with tile.TileContext(nc) as tc:
        with (
            tc.tile_pool(name="const", bufs=1) as cpool,
            tc.tile_pool(name="kvin", bufs=4) as kvpool,
            tc.tile_pool(name="kt", bufs=2) as ktpool,
            tc.tile_pool(name="vt", bufs=4) as vtpool,
            tc.tile_pool(name="attn", bufs=3) as apool,
            tc.tile_pool(name="abT", bufs=17) as tpool,
            tc.tile_pool(name="small", bufs=6) as spool,
            tc.tile_pool(name="oav", bufs=3) as opool,
            tc.tile_pool(name="pp", bufs=2, space="PSUM") as pp,
            tc.tile_pool(name="pl", bufs=2, space="PSUM") as pl,
            tc.tile_pool(name="pav", bufs=2, space="PSUM") as pav,
        ):
            # ---- constants (critical-path loads first, on HWDGE) ----
            w = {}
            for name in ("wq", "wk", "wv", "wo"):
                w[name] = cpool.tile([128, 4, D], bf16, tag=name, name=name)
            qin = cpool.tile([128, 4, N * Q], bf16, tag="qin", name="qin")
            nc.sync.dma_start(out=w["wq"][:], in_=w_aps["wq"][:])
            nc.sync.dma_start(out=qin[:, :, 0:512], in_=qT[:, :, 0:512])
            bq2 = cpool.tile([128, 4], f32, tag="bq2", name="bq2")
            nc.sync.dma_start(out=bq2[:], in_=bq_h.ap()[:])
            bk2 = cpool.tile([128, 4], f32, tag="bk2", name="bk2")
            expb = cpool.tile([Q, KP], bf16, tag="expb", name="expb")
            bob = cpool.tile([128, D], f32, tag="bob", name="bob")

            # ---- Q projection for all steps ----
            qTp = cpool.tile([128, 4, N * Q], bf16, tag="qTp", name="qTp")

            def qproj_unit(jc, mb):
                ps = pp.tile([128, 512], f32, tag="pp", name="pp")
                for ic in range(4):
                    nc.tensor.matmul(
                        ps[:],
                        w["wq"][:, ic, jc * 128:(jc + 1) * 128],
                        qin[:, ic, mb * 512:(mb + 1) * 512],
                        start=(ic == 0), stop=(ic == 3),
                    )
                nc.scalar.activation(
                    out=qTp[:, jc, mb * 512:(mb + 1) * 512], in_=ps[:],
                    func=AF.Identity, bias=bq2[:, jc:jc + 1])

            def load_kv(s, eng):
                n = s % N
                t = kvpool.tile([128, 4, KP], bf16, tag="kvin", name=f"kvin{n}")
                hk = KP // 2
                for half in range(2):
                    eng.dma_start(
                        out=t[:, :, half * hk:(half + 1) * hk],
                        in_=kvT[:, :, n * KP + half * hk:n * KP + (half + 1) * hk])
                return t

            # remaining prologue loads, ordered for the DMA queues: kv0 and
            # the K-proj weights are needed right after Q-proj finishes.
            kv_first = {0: load_kv(0, nc.sync)}
            nc.sync.dma_start(out=qin[:, :, 512:1024], in_=qT[:, :, 512:1024])
            nc.sync.dma_start(out=w["wk"][:], in_=w_aps["wk"][:])
            nc.sync.dma_start(out=bk2[:], in_=bk_h.ap()[:])
            nc.sync.dma_start(out=expb[:], in_=eb_h.ap()[:])
            nc.gpsimd.dma_start(out=w["wv"][:], in_=w_aps["wv"][:])
            kv_first[1] = load_kv(1, nc.gpsimd)

            for mb in range(2):
                for jc in range(4):
                    qproj_unit(jc, mb)

            def kproj_unit(kvin, kt, jc, kb, blocks=KBP):
                kst, ksz = blocks[kb]
                ps = pp.tile([128, 512], f32, tag="pp", name="pp")
                for ic in range(4):
                    nc.tensor.matmul(
                        ps[:, 0:ksz],
                        w["wk"][:, ic, jc * 128:(jc + 1) * 128],
                        kvin[:, ic, kst:kst + ksz],
                        start=(ic == 0), stop=(ic == 3),
                    )
                nc.scalar.activation(
                    out=kt[:, jc, kst:kst + ksz], in_=ps[:, 0:ksz],
                    func=AF.Identity, bias=bk2[:, jc:jc + 1])

            def vproj_unit(kvin, vt, mc):
                # psum from the pav pool: its allocations interleave with AV
                # pairs (~1 slot apart), so a lagging evac never stalls PE
                ps = pav.tile([128, 512], f32, tag="pav", name="psv")
                for ic in range(4):
                    nc.tensor.matmul(
                        ps[:],
                        kvin[:, ic, mc * 128:(mc + 1) * 128],
                        w["wv"][:, ic, :],
                        start=(ic == 0), stop=(ic == 3),
                    )
                nc.vector.tensor_copy(out=vt[:, mc, :], in_=ps[:])

            def kproj(kvin):
                kt = ktpool.tile([128, 4, KP], bf16, tag="kt", name="kt")
                for jc in range(4):
                    for kb in range(len(KBP)):
                        kproj_unit(kvin, kt, jc, kb)
                return kt

            def vproj(kvin):
                vt = vtpool.tile([128, NCH, D], bf16, tag="vt", name="vt")
                for mc in range(NCH):
                    vproj_unit(kvin, vt, mc)
                return vt

            def qk_softmax_pair(n, c, kt):
                """QK for head pair (2c, 2c+1) with row-group-interleaved
                matmuls over the compute width KW; exp on ScalarE; exp-bias
                multiply + fused row-sum on VectorE; per-head normalize on
                VectorE; one shared DMA transpose. The KW:KP pad columns of
                ab are zeroed by GpSimd (idle engine, no chain deps)."""
                psls = [pl.tile([Q, KP], f32, tag="pl", name="pl")
                        for _ in range(2)]
                for kb in range(len(KB)):
                    kst, ksz = KB[kb]
                    for par in range(2):
                        e = par * 64
                        nc.tensor.matmul(
                            psls[par][:, kst:kst + ksz],
                            qTp[e:e + 64, c, n * Q:(n + 1) * Q],
                            kt[e:e + 64, c, kst:kst + ksz],
                            start=True, stop=True,
                        )
                sums = spool.tile([Q, 2], f32, tag="sums", name="sums")
                abs_ = []
                for par in range(2):
                    ae = apool.tile([Q, KP], bf16, tag="ae", bufs=4, name="ae")
                    nc.scalar.activation(out=ae[:, 0:KW],
                                         in_=psls[par][:, 0:KW],
                                         func=AF.Exp)
                    ab = apool.tile([Q, KP], bf16, tag="ab", bufs=6, name="ab")
                    if KW < KP:
                        nc.gpsimd.memset(ab[:, KW:KP], 0.0)
                    nc.vector.scalar_tensor_tensor(
                        out=ab[:, 0:KW], in0=ae[:, 0:KW], scalar=1.0,
                        in1=expb[:, 0:KW],
                        op0=ALU.mult, op1=ALU.mult,
                        accum_out=sums[:, par:par + 1])
                    abs_.append(ab)
                rec = spool.tile([Q, 2], f32, tag="rec", name="rec")
                nc.vector.reciprocal(rec[:], sums[:])
                abn2 = apool.tile([Q, 2 * KP], bf16, tag="abn2", bufs=6,
                                  name="abn2")
                for par in range(2):
                    nc.vector.tensor_scalar_mul(
                        out=abn2[:, par * KP:(par + 1) * KP], in0=abs_[par][:],
                        scalar1=rec[:, par:par + 1])
                abT2 = tpool.tile([128, 2 * NCH, Q], bf16, tag="abT2",
                                  name="abT2")
                nc.sync.dma_start_transpose(abT2[:], abn2[:])
                return abT2

            def av_pair(hc, abT2, vt, oavT, scalar_evac=False):
                """AV for head pair (2hc, 2hc+1), col-group interleaved."""
                psav = pav.tile([128, Q], f32, tag="pav", name="pav")
                for c in range(NCH):
                    for par in range(2):
                        h = 2 * hc + par
                        e = par * 64
                        nc.tensor.matmul(
                            psav[e:e + HD, :],
                            vt[:, c, h * HD:(h + 1) * HD],
                            abT2[:, par * NCH + c, :],
                            start=(c == 0), stop=(c == NCH - 1),
                            skip_group_check=True,
                        )
                if scalar_evac:
                    nc.scalar.activation(out=oavT[:, hc, :], in_=psav[:],
                                         func=AF.Identity)
                else:
                    nc.vector.tensor_copy(out=oavT[:, hc, :], in_=psav[:])

            def outproj(n, oavT):
                pso = pav.tile([Q, D], f32, tag="pav", name="pso")
                for jc in range(4):
                    nc.tensor.matmul(
                        pso[:], oavT[:, jc, :], w["wo"][:, jc, :],
                        start=(jc == 0), stop=(jc == 3))
                osb = opool.tile([Q, D], f32, tag="osb", name="osb")
                nc.vector.scalar_tensor_tensor(
                    out=osb[:], in0=pso[:], scalar=1.0, in1=bob[:],
                    op0=ALU.mult, op1=ALU.add)
                nc.sync.dma_start(out=out_ap[n], in_=osb[:])

            # ---- software-pipelined steps (continuous across repeats) ----
            # K/V-proj lead by 1 step; kv loads lead by 2; AV + out-proj
            # TRAIL by 2 steps so the softmax->transpose chain of step s has
            # a full spare step before AV(s) consumes it at step s+2. The
            # repeat bodies share one pipeline (the drain happens once).
            TOT = repeat * N
            kvs = kv_first
            kts = {0: kproj(kvs[0])}
            vts = {0: vproj(kvs[0])}
            abTs = {}
            for s in range(TOT):
                n = s % N
                if s == 0:
                    nc.gpsimd.dma_start(out=w["wo"][:], in_=w_aps["wo"][:])
                    nc.gpsimd.dma_start(out=bob[:], in_=bo_h.ap()[:])
                last = (s == TOT - 1)
                proj_units = []
                if s + 1 < TOT:
                    ktn = ktpool.tile([128, 4, KP], bf16, tag="kt",
                                      name="kt")
                    vtn = vtpool.tile([128, NCH, D], bf16, tag="vt",
                                      name="vt")
                    proj_units = (
                        [(kproj_unit, (kvs[s + 1], ktn, jc, kb))
                         for jc in range(4) for kb in range(len(KBP))]
                        + [(vproj_unit, (kvs[s + 1], vtn, mc))
                           for mc in range(NCH)])
                    kts[s + 1] = ktn
                    vts[s + 1] = vtn
                oavT_m2 = None
                if s >= 2:
                    oavT_m2 = opool.tile([128, 4, Q], bf16,
                                         tag="oavT", name="oavT")
                oavT_m1 = oavT_l = None
                if last:
                    oavT_m1 = opool.tile([128, 4, Q], bf16,
                                         tag="oavT", name="oavT")
                    oavT_l = opool.tile([128, 4, Q], bf16,
                                        tag="oavT", name="oavT")
                abTs[s] = []
                pu = 0
                npu = len(proj_units)
                for hc in range(4):
                    abTs[s].append(qk_softmax_pair(n, hc, kts[s]))
                    if s >= 2:
                        av_pair(hc, abTs[s - 2][hc], vts[s - 2], oavT_m2)
                    if last:
                        # drain the pipeline: AV(s-1) + staggered AV(s);
                        # evacuate on ScalarE (idle here: no proj units) so
                        # DVE keeps the transpose chain moving
                        av_pair(hc, abTs[s - 1][hc], vts[s - 1], oavT_m1,
                                scalar_evac=True)
                        if hc >= 1:
                            av_pair(hc - 1, abTs[s][hc - 1], vts[s],
                                    oavT_l, scalar_evac=True)
                    take = (npu + 3 - hc) // 4 if hc < 3 else npu - pu
                    for _ in range(max(0, take)):
                        if pu < npu:
                            fn, args = proj_units[pu]
                            fn(*args)
                            pu += 1
                # kv load for s+2 issued at step END so Pool's FIFO serves
                # the ab pad-memsets (which gate the softmax chain) first
                if s + 2 < TOT:
                    kvs[s + 2] = load_kv(s + 2, nc.gpsimd)
                if s >= 2:
                    outproj((s - 2) % N, oavT_m2)
                    abTs[s - 2] = None
                    kvs[s - 2] = kts[s - 2] = vts[s - 2] = None
                if last:
                    outproj((s - 1) % N, oavT_m1)
                    av_pair(3, abTs[s][3], vts[s], oavT_l, scalar_evac=True)
                    outproj(n, oavT_l)

    nc.compile()
    return nc


def _stage_inputs(inputs):
    """Build per-core input maps (host-side sharding + key compaction)."""
    query = np.asarray(inputs["query"], np.float32)
    key_value = np.asarray(inputs["key_value"], np.float32)
    query_pos = np.asarray(inputs["query_pos"], np.float32)
    key_pos = np.asarray(inputs["key_pos"], np.float32)
    key_mask = np.asarray(inputs["key_mask"])
    KP, _ = _kp_from_mask(key_mask)

    wqT = np.ascontiguousarray((np.asarray(inputs["Wq"], np.float32) * SCALE).T
                               ).astype(_BF16)
    wkT = np.ascontiguousarray(np.asarray(inputs["Wk"], np.float32).T).astype(_BF16)
    wvT = np.ascontiguousarray(np.asarray(inputs["Wv"], np.float32).T).astype(_BF16)
    Wo = np.asarray(inputs["Wo"], np.float32)
    woT = np.ascontiguousarray(Wo.T).astype(_BF16)
    bq2 = np.ascontiguousarray(
        (np.asarray(inputs["bq"], np.float32) * SCALE).reshape(4, 128).T)
    bk2 = np.ascontiguousarray(np.asarray(inputs["bk"], np.float32).reshape(4, 128).T)
    # softmax weights sum to 1, so the V-bias contributes bv @ Wo.T to the
    # output regardless of the attention pattern — fold it into bo.
    bo_eff = (np.asarray(inputs["bv"], np.float32) @ Wo.T
              + np.asarray(inputs["bo"], np.float32))
    bob = np.ascontiguousarray(np.broadcast_to(bo_eff, (128, D)))

    in_maps = []
    for b in range(B):
        idx = np.nonzero(key_mask[b])[0]
        nv = len(idx)
        kvc = np.zeros((N, KP, D), np.float32)
        kvc[:, :nv] = key_value[b][:, idx]
        kvT = np.ascontiguousarray(kvc.reshape(N * KP, D).T).astype(_BF16)
        qT = np.ascontiguousarray(query[b].reshape(N * Q, D).T).astype(_BF16)
        kp_c = key_pos[b][idx]
        d2 = ((query_pos[b][:, None, :] - kp_c[None, :, :]) ** 2).sum(-1)
        eb = np.zeros((Q, KP), np.float32)
        eb[:, :nv] = np.exp(-d2 / (2.0 * SIGMA2))
        in_maps.append({
            "kvT": kvT, "qT": qT,
            "wqT": wqT, "wkT": wkT, "wvT": wvT, "woT": woT,
            "bq2": bq2, "bk2": bk2, "bob": bob,
            "expb": eb.astype(_BF16),
        })
    return in_maps


IN_NAMES = ["kvT", "qT", "wqT", "wkT", "wvT", "woT",
            "bq2", "bk2", "bob", "expb"]


def _get_runner(KP, KW):
    """Compile (once) and return a callable in_maps -> list of out arrays."""
    ck = ("runner", KP, KW)
    if ck in _CACHE:
        return _CACHE[ck]

    import jax
    from jax.sharding import Mesh, PartitionSpec
    from jax.experimental.shard_map import shard_map
    from concourse.bass2jax import (_bass_exec_p, install_neuronx_cc_hook,
                                    partition_id_tensor)

    nc = _build_program(KP=KP, KW=KW)
    install_neuronx_cc_hook()

    out_shape = (N, Q, D)
    out_aval = jax.core.ShapedArray(out_shape, np.float32)
    all_names = IN_NAMES + ["out", "partition_id"]

    def _body(*args):
        outs = _bass_exec_p.bind(
            *args, partition_id_tensor(),
            out_avals=(out_aval,),
            in_names=tuple(all_names),
            out_names=("out",),
            lowering_input_output_aliases=(),
            sim_require_finite=True,
            sim_require_nnan=True,
            nc=nc,
        )
        return tuple(outs)

    n_in = len(IN_NAMES)
    devices = jax.devices()[:NCORES]
    mesh = Mesh(np.asarray(devices), ("core",))
    sharded = jax.jit(
        shard_map(_body, mesh=mesh,
                  in_specs=(PartitionSpec("core"),) * (n_in + 1),
                  out_specs=(PartitionSpec("core"),),
                  check_rep=False),
        donate_argnums=(n_in,), keep_unused=True)

    def runner(in_maps):
        concat_in = [np.concatenate([np.asarray(m[name]) for m in in_maps], axis=0)
                     for name in IN_NAMES]
        zeros = np.zeros((NCORES * N, Q, D), np.float32)
        (out,) = sharded(*concat_in, zeros)
        out = np.asarray(out).reshape(NCORES, N, Q, D)
        return out

    _CACHE[ck] = runner
    return runner


def kernel(**inputs):
    KP, KW = _kp_from_mask(np.asarray(inputs["key_mask"]))
    runner = _get_runner(KP, KW)
    in_maps = _stage_inputs(inputs)
    out = runner(in_maps)          # [8 cores = B, N, Q, D]
    return np.ascontiguousarray(out)
